# revision 26
# baseline (speedup 1.0000x reference)
"""ApproxNDCG loss kernel for Trainium2, distributed over 8 NeuronCores.

Strategy (data-parallel over batch dim B=32; 4 rows per core):

For each row (list of L=2048 items) we need
  soft_ranks_i  = 1 + sum_{j != i} sigmoid(p_i - p_j)
  hard_rank0_i  = #{j : t_j > t_i}            (position in descending sort)
  gains_i       = 2^t_i - 1
  approx_dcg    = sum_i gains_i / log2(1 + soft_ranks_i)
  ideal_dcg     = sum_i gains_i / log2(2 + hard_rank0_i)
  loss          = mean_rows(1 - approx_dcg / ideal_dcg)

(the hard-rank formulation of ideal_dcg is exact up to ties, whose
contribution is invariant because tied targets have equal gains).

Engine split per core:
  * ScalarE computes only the upper-triangle trapezoids of the sigmoid
    matrix sigma[i, j] = sigmoid(p_i - p_j) for j >= 128*I (row-chunk I),
    with the free fused accum_out giving the row sums.  The lower
    triangle is reconstructed from antisymmetry:
      sum_{j < 128 I} sigma(p_i - p_j) = 128 I - sum_{j < 128 I} sigma(p_j - p_i)
    where the last sum is a column (partition) reduction of already
    computed trapezoids, done on the TensorEngine as matmuls with a ones
    vector.
  * VectorE computes the hard-rank counts with tensor_scalar(is_gt)
    + fused accumulate over the full row.
  * Per-core partial losses are summed on device; the 8 scalars are
    combined (sum / B) host-side as the unshard step.
"""

import math
from contextlib import ExitStack

import numpy as np

import concourse.bass as bass
import concourse.tile as tile
from concourse import bacc, masks, mybir
from concourse.bass_utils import run_bass_kernel_spmd

B, L = 32, 2048
NCORES = 8
ROWS = B // NCORES          # rows of the batch per core
P = 128                     # SBUF partitions
NCH = L // P                # 16 row-chunks per row
F32 = mybir.dt.float32
LN2 = math.log(2.0)

AF = mybir.ActivationFunctionType
OP = mybir.AluOpType


def _emit(ctx: ExitStack, tc: "tile.TileContext", pred: bass.AP, targ: bass.AP,
          out: bass.AP, dbg: dict | None = None) -> None:
    nc = tc.nc

    rows_pool = ctx.enter_context(tc.tile_pool(name="rows", bufs=2))
    rep_pool = ctx.enter_context(tc.tile_pool(name="rep", bufs=2))
    trap_pool = ctx.enter_context(tc.tile_pool(name="trap", bufs=3))
    cmp_pool = ctx.enter_context(tc.tile_pool(name="cmp", bufs=2))
    small = ctx.enter_context(tc.tile_pool(name="small", bufs=1))
    psum_mir = ctx.enter_context(
        tc.tile_pool(name="mir", bufs=ROWS, space="PSUM"))
    psum_red = ctx.enter_context(tc.tile_pool(name="red", bufs=1, space="PSUM"))
    psum_tp = ctx.enter_context(tc.tile_pool(name="tp", bufs=2, space="PSUM"))

    # --- constants -----------------------------------------------------
    ones_col = small.tile([P, 1], F32, tag="ones_col")
    nc.vector.memset(ones_col[:], 1.0)
    two_col = small.tile([P, 1], F32, tag="two_col")
    nc.vector.memset(two_col[:], 2.0)
    ident = small.tile([NCH, NCH], F32, tag="ident")
    masks.make_identity(nc, ident[:])
    # per-chunk constant: soft-rank Ln argument offset  128*I + 1.5
    const_cols = small.tile([P, NCH], F32, tag="const_cols")
    for I in range(NCH):
        nc.vector.memset(const_cols[:, I:I + 1], 128.0 * I + 1.5)

    # persistent per-row stats, rows side by side in the free dim
    pT_all = small.tile([P, NCH * ROWS], F32, tag="pT_all")
    tT_all = small.tile([P, NCH * ROWS], F32, tag="tT_all")
    sig_all = small.tile([P, NCH * ROWS], F32, tag="sig_all")
    cnt_all = small.tile([P, NCH * ROWS], F32, tag="cnt_all")
    # numerator sums in cols [0, ROWS), denominator sums in [ROWS, 2*ROWS)
    acc_all = small.tile([P, 2 * ROWS], F32, tag="acc_all")

    # [16, 128] row views for the PE chunk-transpose: pT[q, f] = p[128 f + q]
    predC = pred.rearrange("b (a c) -> b a c", a=NCH)
    targC = targ.rearrange("b (a c) -> b a c", a=NCH)

    mirs = []
    for r in range(ROWS):
        pT = pT_all[:, r * NCH:(r + 1) * NCH]
        tT = tT_all[:, r * NCH:(r + 1) * NCH]
        sig_acc = sig_all[:, r * NCH:(r + 1) * NCH]
        counts = cnt_all[:, r * NCH:(r + 1) * NCH]

        p_row = rows_pool.tile([1, L], F32, tag="p_row")
        nc.sync.dma_start(p_row[:], pred[r:r + 1, :])
        t_row = rows_pool.tile([1, L], F32, tag="t_row")
        nc.sync.dma_start(t_row[:], targ[r:r + 1, :])
        for src, dst in ((predC, pT), (targC, tT)):
            c16 = rows_pool.tile([NCH, P], F32, tag="c16")
            nc.sync.dma_start(c16[:], src[r])
            tp = psum_tp.tile([P, NCH], F32, tag="tp")
            nc.tensor.transpose(tp[:], c16[:], ident[:])
            nc.vector.tensor_copy(dst, tp[:])

        p_rep = rep_pool.tile([P, L], F32, tag="p_rep")
        nc.gpsimd.partition_broadcast(p_rep[:], p_row[:])
        t_rep = rep_pool.tile([P, L], F32, tag="t_rep")
        nc.gpsimd.partition_broadcast(t_rep[:], t_row[:])

        mir = psum_mir.tile([P, NCH], F32, tag="mir")
        mirs.append(mir)
        nc.vector.memset(mir[:, 0:1], 0.0)

        for I in range(NCH):
            W = L - P * I
            # sigma[i, j] = sigmoid(p_i - p_j), i = 128 I + q, j >= 128 I
            trap = trap_pool.tile([P, L], F32, tag="trap")
            nc.scalar.activation(
                trap[:, :W], p_rep[:, P * I:], AF.Sigmoid,
                bias=pT[:, I:I + 1], scale=-1.0,
                accum_out=sig_acc[:, I:I + 1])
            # column sums of this trapezoid feed the mirror term of every
            # later chunk I2: mir[q, I2] += sum_q' trap[q', 128 (I2 - I) + q]
            # The whole mir bank is ONE accumulation group: start on the
            # first matmul pending-zeroes the full 2KB zero region, so each
            # column's first contribution overwrites and later ones add.
            for I2 in range(I + 1, NCH):
                nc.tensor.matmul(
                    mir[:, I2:I2 + 1],
                    lhsT=trap[:, P * (I2 - I):P * (I2 - I) + P],
                    rhs=ones_col[:],
                    start=(I == 0 and I2 == 1),
                    stop=(I == NCH - 2 and I2 == NCH - 1),
                    skip_group_check=True)
            # hard-rank counts: counts[q, I] = #{j : t_j > t_i}
            cmpt = cmp_pool.tile([P, L], F32, tag="cmp")
            nc.vector.tensor_scalar(
                cmpt[:], t_rep[:], tT[:, I:I + 1], None,
                op0=OP.is_gt, op1=OP.add,
                accum_out=counts[:, I:I + 1])

    # --- epilogue (all transcendentals grouped to batch ACT table sets) ---
    for r in range(ROWS):
        pT = pT_all[:, r * NCH:(r + 1) * NCH]
        tT = tT_all[:, r * NCH:(r + 1) * NCH]
        sig_acc = sig_all[:, r * NCH:(r + 1) * NCH]
        counts = cnt_all[:, r * NCH:(r + 1) * NCH]
        mir = mirs[r]

        # soft Ln argument: 1 + soft_rank = sig_acc - mir + (128 I + 1.5)
        s1 = small.tile([P, NCH], F32, tag="s1")
        nc.vector.tensor_tensor(s1[:], sig_acc, mir[:], op=OP.subtract)
        s2 = small.tile([P, NCH], F32, tag="s2")
        nc.vector.tensor_tensor(s2[:], s1[:], const_cols[:], op=OP.add)
        ln_s = small.tile([P, NCH], F32, tag="ln_s")
        nc.scalar.activation(ln_s[:], s2[:], AF.Ln)
        # ideal Ln argument: hard_rank0 + 2
        ln_i = small.tile([P, NCH], F32, tag="ln_i")
        nc.scalar.activation(ln_i[:], counts, AF.Ln, bias=two_col[:])
        # gains: 2^t - 1
        g = small.tile([P, NCH], F32, tag="g")
        nc.scalar.activation(g[:], tT, AF.Exp, scale=LN2)
        gm1 = small.tile([P, NCH], F32, tag="gm1")
        nc.vector.tensor_scalar(gm1[:], g[:], 1.0, None, op0=OP.subtract)

        inv_s = small.tile([P, NCH], F32, tag="inv_s")
        nc.vector.reciprocal(inv_s[:], ln_s[:])
        inv_i = small.tile([P, NCH], F32, tag="inv_i")
        nc.vector.reciprocal(inv_i[:], ln_i[:])

        prod_a = small.tile([P, NCH], F32, tag="prod_a")
        nc.vector.tensor_tensor(prod_a[:], gm1[:], inv_s[:], op=OP.mult)
        nc.vector.reduce_sum(acc_all[:, r:r + 1], prod_a[:],
                             axis=mybir.AxisListType.X)
        prod_b = small.tile([P, NCH], F32, tag="prod_b")
        nc.vector.tensor_tensor(prod_b[:], gm1[:], inv_i[:], op=OP.mult)
        nc.vector.reduce_sum(acc_all[:, ROWS + r:ROWS + r + 1], prod_b[:],
                             axis=mybir.AxisListType.X)

    if dbg is not None:
        nc.sync.dma_start(dbg["sig"][:, :], sig_all[:])
        nc.sync.dma_start(dbg["cnt"][:, :], cnt_all[:])
        for r in range(ROWS):
            mcopy = small.tile([P, NCH], F32, tag=f"mcopy{r}")
            nc.vector.tensor_copy(mcopy[:], mirs[r][:])
            nc.sync.dma_start(dbg["mir"][:, r * NCH:(r + 1) * NCH], mcopy[:])
        nc.sync.dma_start(dbg["acc"][:, :], acc_all[:])

    # partition-reduce the per-partition partial sums: [128, R] -> [R, 1]
    # (lhsT = acc columns so M = ROWS; an M=1 ones-lhsT matmul miscomputes
    # on this silicon)
    num_red = psum_red.tile([ROWS, 1], F32, tag="num_red")
    nc.tensor.matmul(num_red[:], lhsT=acc_all[:, 0:ROWS], rhs=ones_col[:],
                     start=True, stop=True)
    den_red = psum_red.tile([ROWS, 1], F32, tag="den_red")
    nc.tensor.matmul(den_red[:], lhsT=acc_all[:, ROWS:2 * ROWS],
                     rhs=ones_col[:], start=True, stop=True)

    num_sb = small.tile([ROWS, 1], F32, tag="num_sb")
    nc.vector.tensor_copy(num_sb[:], num_red[:])
    den_sb = small.tile([ROWS, 1], F32, tag="den_sb")
    nc.vector.tensor_copy(den_sb[:], den_red[:])
    inv_den = small.tile([ROWS, 1], F32, tag="inv_den")
    nc.vector.reciprocal(inv_den[:], den_sb[:])
    ratio = small.tile([ROWS, 1], F32, tag="ratio")
    nc.vector.tensor_tensor(ratio[:], num_sb[:], inv_den[:], op=OP.mult)
    rowloss = small.tile([ROWS, 1], F32, tag="rowloss")
    nc.vector.tensor_scalar(rowloss[:], ratio[:], -1.0, 1.0,
                            op0=OP.mult, op1=OP.add)
    nc.sync.dma_start(out[:, :], rowloss[:])


def build(debug: bool = False) -> bass.Bass:
    nc = bacc.Bacc(trn_type="TRN2")
    pred = nc.dram_tensor("predictions", [ROWS, L], F32, kind="ExternalInput")
    targ = nc.dram_tensor("targets", [ROWS, L], F32, kind="ExternalInput")
    out = nc.dram_tensor("out", [ROWS, 1], F32, kind="ExternalOutput")
    dbg = None
    if debug:
        dbg = {
            "sig": nc.dram_tensor("dbg_sig", [P, NCH * ROWS], F32,
                                  kind="ExternalOutput").ap(),
            "cnt": nc.dram_tensor("dbg_cnt", [P, NCH * ROWS], F32,
                                  kind="ExternalOutput").ap(),
            "mir": nc.dram_tensor("dbg_mir", [P, NCH * ROWS], F32,
                                  kind="ExternalOutput").ap(),
            "acc": nc.dram_tensor("dbg_acc", [P, 2 * ROWS], F32,
                                  kind="ExternalOutput").ap(),
        }
    with tile.TileContext(nc) as tc:
        with ExitStack() as ctx:
            _emit(ctx, tc, pred.ap(), targ.ap(), out.ap(), dbg)
    nc.compile()
    return nc


def make_in_maps(predictions: np.ndarray, targets: np.ndarray):
    predictions = np.ascontiguousarray(predictions, dtype=np.float32)
    targets = np.ascontiguousarray(targets, dtype=np.float32)
    return [
        {
            "predictions": predictions[c * ROWS:(c + 1) * ROWS],
            "targets": targets[c * ROWS:(c + 1) * ROWS],
        }
        for c in range(NCORES)
    ]


def kernel(predictions: np.ndarray, targets: np.ndarray, _trace: bool = False,
           **_run_kwargs):
    nc = build()
    in_maps = make_in_maps(predictions, targets)
    res = run_bass_kernel_spmd(nc, in_maps, core_ids=list(range(NCORES)),
                               trace=_trace, **_run_kwargs)
    partial = sum(float(r["out"][:, 0].sum()) for r in res.results)
    loss = np.float32(partial / B)
    if _trace:
        return np.asarray(loss), res
    return np.asarray(loss)


# revision 27
# speedup vs baseline: 1.6524x; 1.6524x over previous
"""ApproxNDCG loss kernel for Trainium2, distributed over 8 NeuronCores.

Strategy (data-parallel over batch dim B=32; 4 rows per core):

For each row (list of L=2048 items):
  soft_ranks_i  = 1 + sum_{j != i} sigmoid(p_i - p_j)
  hard_rank0_i  = #{j : t_j > t_i}            (position in descending sort)
  gains_i       = 2^t_i - 1
  approx_dcg    = sum_i gains_i / log2(1 + soft_ranks_i)
  ideal_dcg     = sum_i gains_i / log2(2 + hard_rank0_i)
  loss          = mean_rows(1 - approx_dcg / ideal_dcg)

(the hard-rank formulation of ideal_dcg is exact up to float ties, whose
contribution is invariant because tied targets have equal gains).

Both pairwise matrices are only computed on their upper triangle
(row-chunk trapezoids, j >= 128*I), in bf16, with the fused per-partition
accum_out giving the direct row sums.  The lower triangle is
reconstructed from (anti)symmetry with TensorEngine column reductions of
the already-computed trapezoids (bf16 weights -> fast LDWEIGHTS):
  sigma(p_i - p_j) = 1 - sigma(p_j - p_i)
  (t_j > t_i)      = 1 - (t_i > t_j)   (up to measure-zero ties)
ScalarE computes sigmoid trapezoids; VectorE computes is_gt trapezoids;
TensorE folds both mirror terms into one PSUM bank per row.
"""

import math
from contextlib import ExitStack

import numpy as np

import concourse.bass as bass
import concourse.tile as tile
from concourse import bacc, masks, mybir
from concourse.bass_utils import run_bass_kernel_spmd

B, L = 32, 2048
NCORES = 8
ROWS = B // NCORES          # rows of the batch per core
P = 128                     # SBUF partitions
NCH = L // P                # 16 row-chunks per row
F32 = mybir.dt.float32
BF16 = mybir.dt.bfloat16
LN2 = math.log(2.0)

AF = mybir.ActivationFunctionType
OP = mybir.AluOpType


def _emit(ctx: ExitStack, tc: "tile.TileContext", pred: bass.AP, targ: bass.AP,
          out: bass.AP, dbg: dict | None = None) -> None:
    nc = tc.nc

    rows_pool = ctx.enter_context(tc.tile_pool(name="rows", bufs=2))
    rep_pool = ctx.enter_context(tc.tile_pool(name="rep", bufs=2))
    trap_pool = ctx.enter_context(tc.tile_pool(name="trap", bufs=3))
    cmp_pool = ctx.enter_context(tc.tile_pool(name="cmp", bufs=3))
    small = ctx.enter_context(tc.tile_pool(name="small", bufs=1))
    psum_mir = ctx.enter_context(
        tc.tile_pool(name="mir", bufs=ROWS, space="PSUM"))
    psum_red = ctx.enter_context(tc.tile_pool(name="red", bufs=1, space="PSUM"))
    psum_tp = ctx.enter_context(tc.tile_pool(name="tp", bufs=2, space="PSUM"))

    # --- constants -----------------------------------------------------
    ones_bf = small.tile([P, 1], BF16, tag="ones_bf")
    nc.vector.memset(ones_bf[:], 1.0)
    ones_col = small.tile([P, 1], F32, tag="ones_col")
    nc.vector.memset(ones_col[:], 1.0)
    two_col = small.tile([P, 1], F32, tag="two_col")
    nc.vector.memset(two_col[:], 2.0)
    ident = small.tile([NCH, NCH], F32, tag="ident")
    masks.make_identity(nc, ident[:])
    # per-chunk offsets: soft Ln arg 128 I + 1.5, ideal Ln arg 128 I + 2
    const_soft = small.tile([P, NCH], F32, tag="const_soft")
    const_ideal = small.tile([P, NCH], F32, tag="const_ideal")
    for I in range(NCH):
        nc.vector.memset(const_soft[:, I:I + 1], 128.0 * I + 1.5)
        nc.vector.memset(const_ideal[:, I:I + 1], 128.0 * I + 2.0)

    # persistent per-row stats, rows side by side in the free dim
    pT_all = small.tile([P, NCH * ROWS], F32, tag="pT_all")
    tT_all = small.tile([P, NCH * ROWS], F32, tag="tT_all")
    sig_all = small.tile([P, NCH * ROWS], F32, tag="sig_all")
    cnt_all = small.tile([P, NCH * ROWS], F32, tag="cnt_all")
    gm1_all = small.tile([P, NCH * ROWS], F32, tag="gm1_all")
    # numerator sums in cols [0, ROWS), denominator sums in [ROWS, 2*ROWS)
    acc_all = small.tile([P, 2 * ROWS], F32, tag="acc_all")

    # [16, 128] row views for the PE chunk-transpose: pT[q, f] = p[128 f + q]
    predC = pred.rearrange("b (a c) -> b a c", a=NCH)
    targC = targ.rearrange("b (a c) -> b a c", a=NCH)

    # --- phase A: loads, transposes, gains (all Exp ACTs batched) ------
    p_rows, t_rows = [], []
    for r in range(ROWS):
        pT = pT_all[:, r * NCH:(r + 1) * NCH]
        tT = tT_all[:, r * NCH:(r + 1) * NCH]
        p_row = rows_pool.tile([1, L], F32, tag=f"p_row{r}")
        nc.sync.dma_start(p_row[:], pred[r:r + 1, :])
        t_row = rows_pool.tile([1, L], F32, tag=f"t_row{r}")
        nc.sync.dma_start(t_row[:], targ[r:r + 1, :])
        p_rows.append(p_row)
        t_rows.append(t_row)
        for src, dst in ((predC, pT), (targC, tT)):
            c16 = rows_pool.tile([NCH, P], F32, tag="c16")
            nc.sync.dma_start(c16[:], src[r])
            tp = psum_tp.tile([P, NCH], F32, tag="tp")
            nc.tensor.transpose(tp[:], c16[:], ident[:])
            nc.vector.tensor_copy(dst, tp[:])
        # gains: 2^t - 1 (Exp batched before any Sigmoid table load)
        g = small.tile([P, NCH], F32, tag="g")
        nc.scalar.activation(g[:], tT, AF.Exp, scale=LN2)
        nc.vector.tensor_scalar(gm1_all[:, r * NCH:(r + 1) * NCH], g[:],
                                1.0, None, op0=OP.subtract)

    # --- phase B: pairwise trapezoids + mirror column sums -------------
    mirs = []
    for r in range(ROWS):
        pT = pT_all[:, r * NCH:(r + 1) * NCH]
        tT = tT_all[:, r * NCH:(r + 1) * NCH]
        sig_acc = sig_all[:, r * NCH:(r + 1) * NCH]
        cnt_acc = cnt_all[:, r * NCH:(r + 1) * NCH]

        p_rep = rep_pool.tile([P, L], F32, tag="p_rep")
        nc.gpsimd.partition_broadcast(p_rep[:], p_rows[r][:])
        t_rep = rep_pool.tile([P, L], F32, tag="t_rep")
        nc.gpsimd.partition_broadcast(t_rep[:], t_rows[r][:])

        # one PSUM bank per row: cols [0,16) sigma-mirror, [16,32) cnt-mirror
        mir = psum_mir.tile([P, 2 * NCH], F32, tag="mir")
        mirs.append(mir)
        nc.vector.memset(mir[:, 0:1], 0.0)
        nc.vector.memset(mir[:, NCH:NCH + 1], 0.0)

        for I in range(NCH):
            W = L - P * I
            # sigma[i, j] = sigmoid(p_i - p_j), i = 128 I + q, j >= 128 I
            trap = trap_pool.tile([P, L], BF16, tag="trap")
            nc.scalar.activation(
                trap[:, :W], p_rep[:, P * I:], AF.Sigmoid,
                bias=pT[:, I:I + 1], scale=-1.0,
                accum_out=sig_acc[:, I:I + 1])
            # cmp[i, j] = (t_j > t_i)
            cmpt = cmp_pool.tile([P, L], BF16, tag="cmp")
            nc.vector.tensor_scalar(
                cmpt[:, :W], t_rep[:, P * I:], tT[:, I:I + 1], None,
                op0=OP.is_gt, op1=OP.add,
                accum_out=cnt_acc[:, I:I + 1])
            # mirror column sums; the whole mir bank is ONE accumulation
            # group (start pending-zeroes the 2KB zero region, so each
            # column's first contribution overwrites and later ones add)
            for I2 in range(I + 1, NCH):
                o = P * (I2 - I)
                nc.tensor.matmul(
                    mir[:, I2:I2 + 1],
                    lhsT=trap[:, o:o + P], rhs=ones_bf[:],
                    start=(I == 0 and I2 == 1), stop=False,
                    skip_group_check=True)
                nc.tensor.matmul(
                    mir[:, NCH + I2:NCH + I2 + 1],
                    lhsT=cmpt[:, o:o + P], rhs=ones_bf[:],
                    start=False,
                    stop=(I == NCH - 2 and I2 == NCH - 1),
                    skip_group_check=True)

    # --- phase C: epilogue (Ln ACTs batched) ---------------------------
    for r in range(ROWS):
        sig_acc = sig_all[:, r * NCH:(r + 1) * NCH]
        cnt_acc = cnt_all[:, r * NCH:(r + 1) * NCH]
        gm1 = gm1_all[:, r * NCH:(r + 1) * NCH]
        mir = mirs[r]

        # soft Ln argument: 1 + soft_rank
        #   = sig_acc + (128 I - mir_sig) + 0.5 + 1
        s1 = small.tile([P, NCH], F32, tag="s1")
        nc.vector.tensor_tensor(s1[:], sig_acc, mir[:, 0:NCH], op=OP.subtract)
        s2 = small.tile([P, NCH], F32, tag="s2")
        nc.vector.tensor_tensor(s2[:], s1[:], const_soft[:], op=OP.add)
        ln_s = small.tile([P, NCH], F32, tag="ln_s")
        nc.scalar.activation(ln_s[:], s2[:], AF.Ln)
        # ideal Ln argument: hard_rank0 + 2
        #   = cnt_acc + (128 I - mir_cnt) + 2
        s3 = small.tile([P, NCH], F32, tag="s3")
        nc.vector.tensor_tensor(s3[:], cnt_acc, mir[:, NCH:2 * NCH],
                                op=OP.subtract)
        s4 = small.tile([P, NCH], F32, tag="s4")
        nc.vector.tensor_tensor(s4[:], s3[:], const_ideal[:], op=OP.add)
        ln_i = small.tile([P, NCH], F32, tag="ln_i")
        nc.scalar.activation(ln_i[:], s4[:], AF.Ln)

        inv_s = small.tile([P, NCH], F32, tag="inv_s")
        nc.vector.reciprocal(inv_s[:], ln_s[:])
        inv_i = small.tile([P, NCH], F32, tag="inv_i")
        nc.vector.reciprocal(inv_i[:], ln_i[:])

        prod_a = small.tile([P, NCH], F32, tag="prod_a")
        nc.vector.tensor_tensor(prod_a[:], gm1, inv_s[:], op=OP.mult)
        nc.vector.reduce_sum(acc_all[:, r:r + 1], prod_a[:],
                             axis=mybir.AxisListType.X)
        prod_b = small.tile([P, NCH], F32, tag="prod_b")
        nc.vector.tensor_tensor(prod_b[:], gm1, inv_i[:], op=OP.mult)
        nc.vector.reduce_sum(acc_all[:, ROWS + r:ROWS + r + 1], prod_b[:],
                             axis=mybir.AxisListType.X)

    if dbg is not None:
        nc.sync.dma_start(dbg["sig"][:, :], sig_all[:])
        nc.sync.dma_start(dbg["cnt"][:, :], cnt_all[:])
        for r in range(ROWS):
            mcopy = small.tile([P, 2 * NCH], F32, tag=f"mcopy{r}")
            nc.vector.tensor_copy(mcopy[:], mirs[r][:])
            nc.sync.dma_start(dbg["mir"][:, r * 2 * NCH:(r + 1) * 2 * NCH],
                              mcopy[:])
        nc.sync.dma_start(dbg["acc"][:, :], acc_all[:])

    # partition-reduce the per-partition partial sums: [128, R] -> [R, 1]
    # (lhsT = acc columns so M = ROWS; an M=1 ones-lhsT matmul is avoided)
    num_red = psum_red.tile([ROWS, 1], F32, tag="num_red")
    nc.tensor.matmul(num_red[:], lhsT=acc_all[:, 0:ROWS], rhs=ones_col[:],
                     start=True, stop=True)
    den_red = psum_red.tile([ROWS, 1], F32, tag="den_red")
    nc.tensor.matmul(den_red[:], lhsT=acc_all[:, ROWS:2 * ROWS],
                     rhs=ones_col[:], start=True, stop=True)

    num_sb = small.tile([ROWS, 1], F32, tag="num_sb")
    nc.vector.tensor_copy(num_sb[:], num_red[:])
    den_sb = small.tile([ROWS, 1], F32, tag="den_sb")
    nc.vector.tensor_copy(den_sb[:], den_red[:])
    inv_den = small.tile([ROWS, 1], F32, tag="inv_den")
    nc.vector.reciprocal(inv_den[:], den_sb[:])
    ratio = small.tile([ROWS, 1], F32, tag="ratio")
    nc.vector.tensor_tensor(ratio[:], num_sb[:], inv_den[:], op=OP.mult)
    rowloss = small.tile([ROWS, 1], F32, tag="rowloss")
    nc.vector.tensor_scalar(rowloss[:], ratio[:], -1.0, 1.0,
                            op0=OP.mult, op1=OP.add)
    nc.sync.dma_start(out[:, :], rowloss[:])


def build(debug: bool = False) -> bass.Bass:
    nc = bacc.Bacc(trn_type="TRN2")
    pred = nc.dram_tensor("predictions", [ROWS, L], F32, kind="ExternalInput")
    targ = nc.dram_tensor("targets", [ROWS, L], F32, kind="ExternalInput")
    out = nc.dram_tensor("out", [ROWS, 1], F32, kind="ExternalOutput")
    dbg = None
    if debug:
        dbg = {
            "sig": nc.dram_tensor("dbg_sig", [P, NCH * ROWS], F32,
                                  kind="ExternalOutput").ap(),
            "cnt": nc.dram_tensor("dbg_cnt", [P, NCH * ROWS], F32,
                                  kind="ExternalOutput").ap(),
            "mir": nc.dram_tensor("dbg_mir", [P, 2 * NCH * ROWS], F32,
                                  kind="ExternalOutput").ap(),
            "acc": nc.dram_tensor("dbg_acc", [P, 2 * ROWS], F32,
                                  kind="ExternalOutput").ap(),
        }
    with tile.TileContext(nc) as tc:
        with ExitStack() as ctx:
            _emit(ctx, tc, pred.ap(), targ.ap(), out.ap(), dbg)
    nc.compile()
    return nc


def make_in_maps(predictions: np.ndarray, targets: np.ndarray):
    predictions = np.ascontiguousarray(predictions, dtype=np.float32)
    targets = np.ascontiguousarray(targets, dtype=np.float32)
    return [
        {
            "predictions": predictions[c * ROWS:(c + 1) * ROWS],
            "targets": targets[c * ROWS:(c + 1) * ROWS],
        }
        for c in range(NCORES)
    ]


def kernel(predictions: np.ndarray, targets: np.ndarray, _trace: bool = False,
           **_run_kwargs):
    nc = build()
    in_maps = make_in_maps(predictions, targets)
    res = run_bass_kernel_spmd(nc, in_maps, core_ids=list(range(NCORES)),
                               trace=_trace, **_run_kwargs)
    partial = sum(float(r["out"][:, 0].sum()) for r in res.results)
    loss = np.float32(partial / B)
    if _trace:
        return np.asarray(loss), res
    return np.asarray(loss)


# revision 28
# speedup vs baseline: 2.0024x; 1.2118x over previous
"""ApproxNDCG loss kernel for Trainium2, distributed over 8 NeuronCores.

Strategy (data-parallel over batch dim B=32; 4 rows per core):

For each row (list of L=2048 items):
  soft_ranks_i  = 1 + sum_{j != i} sigmoid(p_i - p_j)
  hard_rank0_i  = #{j : t_j > t_i}            (position in descending sort)
  gains_i       = 2^t_i - 1
  approx_dcg    = sum_i gains_i / log2(1 + soft_ranks_i)
  ideal_dcg     = sum_i gains_i / log2(2 + hard_rank0_i)
  loss          = mean_rows(1 - approx_dcg / ideal_dcg)

(the hard-rank formulation of ideal_dcg is exact up to float ties, whose
contribution is invariant because tied targets have equal gains).

Both pairwise matrices are only computed on their upper triangle
(row-chunk trapezoids, j >= 128*I), in bf16, with the fused per-partition
accum_out giving the direct row sums.  The lower triangle is
reconstructed from (anti)symmetry with TensorEngine column reductions of
the already-computed trapezoids (bf16 weights -> fast LDWEIGHTS):
  sigma(p_i - p_j) = 1 - sigma(p_j - p_i)
  (t_j > t_i)      = 1 - (t_i > t_j)   (up to measure-zero ties)
ScalarE computes sigmoid trapezoids; VectorE computes is_gt trapezoids;
TensorE folds both mirror terms into one PSUM bank per row.
"""

import math
from contextlib import ExitStack

import numpy as np

import concourse.bass as bass
import concourse.tile as tile
from concourse import bacc, masks, mybir
from concourse.bass_utils import run_bass_kernel_spmd

B, L = 32, 2048
NCORES = 8
ROWS = B // NCORES          # rows of the batch per core
P = 128                     # SBUF partitions
NCH = L // P                # 16 row-chunks per row
F32 = mybir.dt.float32
BF16 = mybir.dt.bfloat16
LN2 = math.log(2.0)

AF = mybir.ActivationFunctionType
OP = mybir.AluOpType


def _emit(ctx: ExitStack, tc: "tile.TileContext", pred: bass.AP, targ: bass.AP,
          out: bass.AP, dbg: dict | None = None) -> None:
    nc = tc.nc

    rows_pool = ctx.enter_context(tc.tile_pool(name="rows", bufs=2))
    rep_pool = ctx.enter_context(tc.tile_pool(name="rep", bufs=2))
    trap_pool = ctx.enter_context(tc.tile_pool(name="trap", bufs=3))
    cmp_pool = ctx.enter_context(tc.tile_pool(name="cmp", bufs=3))
    small = ctx.enter_context(tc.tile_pool(name="small", bufs=1))
    psum_mir = ctx.enter_context(
        tc.tile_pool(name="mir", bufs=ROWS, space="PSUM"))
    psum_red = ctx.enter_context(tc.tile_pool(name="red", bufs=1, space="PSUM"))
    psum_tp = ctx.enter_context(tc.tile_pool(name="tp", bufs=2, space="PSUM"))

    # --- constants -----------------------------------------------------
    ones_bf = small.tile([P, 1], BF16, tag="ones_bf")
    nc.vector.memset(ones_bf[:], 1.0)
    ones_col = small.tile([P, 1], F32, tag="ones_col")
    nc.vector.memset(ones_col[:], 1.0)
    two_col = small.tile([P, 1], F32, tag="two_col")
    nc.vector.memset(two_col[:], 2.0)
    ident = small.tile([NCH, NCH], F32, tag="ident")
    masks.make_identity(nc, ident[:])
    # per-chunk offsets: soft Ln arg 128 I + 1.5, ideal Ln arg 128 I + 2
    const_soft = small.tile([P, NCH], F32, tag="const_soft")
    const_ideal = small.tile([P, NCH], F32, tag="const_ideal")
    for I in range(NCH):
        nc.vector.memset(const_soft[:, I:I + 1], 128.0 * I + 1.5)
        nc.vector.memset(const_ideal[:, I:I + 1], 128.0 * I + 2.0)

    # persistent per-row stats, rows side by side in the free dim
    pT_all = small.tile([P, NCH * ROWS], F32, tag="pT_all")
    tT_all = small.tile([P, NCH * ROWS], F32, tag="tT_all")
    sig_all = small.tile([P, NCH * ROWS], F32, tag="sig_all")
    cnt_all = small.tile([P, NCH * ROWS], F32, tag="cnt_all")
    gm1_all = small.tile([P, NCH * ROWS], F32, tag="gm1_all")
    # numerator sums in cols [0, ROWS), denominator sums in [ROWS, 2*ROWS)
    acc_all = small.tile([P, 2 * ROWS], F32, tag="acc_all")

    # [16, 128] row views for the PE chunk-transpose: pT[q, f] = p[128 f + q]
    predC = pred.rearrange("b (a c) -> b a c", a=NCH)
    targC = targ.rearrange("b (a c) -> b a c", a=NCH)

    # --- phase A: loads, transposes, gains (all Exp ACTs batched).
    # Order matters: all tT copies land before any Exp is runnable, and
    # the gm1 subtracts come after every Exp, so the ACT queue runs the 4
    # Exps back-to-back (one exp table load) before the sigmoid stream.
    p_rows, t_rows = [], []
    for r in range(ROWS):
        pT = pT_all[:, r * NCH:(r + 1) * NCH]
        tT = tT_all[:, r * NCH:(r + 1) * NCH]
        p_row = rows_pool.tile([1, L], F32, tag=f"p_row{r}")
        nc.sync.dma_start(p_row[:], pred[r:r + 1, :])
        t_row = rows_pool.tile([1, L], F32, tag=f"t_row{r}")
        nc.sync.dma_start(t_row[:], targ[r:r + 1, :])
        p_rows.append(p_row)
        t_rows.append(t_row)
        for src, dst in ((predC, pT), (targC, tT)):
            c16 = rows_pool.tile([NCH, P], F32, tag="c16")
            nc.sync.dma_start(c16[:], src[r])
            tp = psum_tp.tile([P, NCH], F32, tag="tp")
            nc.tensor.transpose(tp[:], c16[:], ident[:])
            nc.vector.tensor_copy(dst, tp[:])
    gs = []
    for r in range(ROWS):
        g = small.tile([P, NCH], F32, tag=f"g{r}")
        nc.scalar.activation(g[:], tT_all[:, r * NCH:(r + 1) * NCH],
                             AF.Exp, scale=LN2)
        gs.append(g)
    for r in range(ROWS):
        nc.vector.tensor_scalar(gm1_all[:, r * NCH:(r + 1) * NCH], gs[r][:],
                                1.0, None, op0=OP.subtract)

    # --- phase B: pairwise trapezoids + mirror column sums -------------
    mirs = []
    for r in range(ROWS):
        pT = pT_all[:, r * NCH:(r + 1) * NCH]
        tT = tT_all[:, r * NCH:(r + 1) * NCH]
        sig_acc = sig_all[:, r * NCH:(r + 1) * NCH]
        cnt_acc = cnt_all[:, r * NCH:(r + 1) * NCH]

        p_rep = rep_pool.tile([P, L], F32, tag="p_rep")
        nc.gpsimd.partition_broadcast(p_rep[:], p_rows[r][:])
        t_rep = rep_pool.tile([P, L], F32, tag="t_rep")
        nc.gpsimd.partition_broadcast(t_rep[:], t_rows[r][:])

        # one PSUM bank per row: cols [0,16) sigma-mirror, [16,32) cnt-mirror
        mir = psum_mir.tile([P, 2 * NCH], F32, tag="mir")
        mirs.append(mir)
        nc.vector.memset(mir[:, 0:1], 0.0)
        nc.vector.memset(mir[:, NCH:NCH + 1], 0.0)

        for I in range(NCH):
            W = L - P * I
            # sigma[i, j] = sigmoid(p_i - p_j), i = 128 I + q, j >= 128 I
            trap = trap_pool.tile([P, L], BF16, tag="trap")
            nc.scalar.activation(
                trap[:, :W], p_rep[:, P * I:], AF.Sigmoid,
                bias=pT[:, I:I + 1], scale=-1.0,
                accum_out=sig_acc[:, I:I + 1])
            # cmp[i, j] = (t_j > t_i)
            cmpt = cmp_pool.tile([P, L], BF16, tag="cmp")
            nc.vector.tensor_scalar(
                cmpt[:, :W], t_rep[:, P * I:], tT[:, I:I + 1], None,
                op0=OP.is_gt, op1=OP.add,
                accum_out=cnt_acc[:, I:I + 1])
            # mirror column sums; the whole mir bank is ONE accumulation
            # group (start pending-zeroes the 2KB zero region, so each
            # column's first contribution overwrites and later ones add)
            for I2 in range(I + 1, NCH):
                o = P * (I2 - I)
                nc.tensor.matmul(
                    mir[:, I2:I2 + 1],
                    lhsT=trap[:, o:o + P], rhs=ones_bf[:],
                    start=(I == 0 and I2 == 1), stop=False,
                    skip_group_check=True)
                nc.tensor.matmul(
                    mir[:, NCH + I2:NCH + I2 + 1],
                    lhsT=cmpt[:, o:o + P], rhs=ones_bf[:],
                    start=False,
                    stop=(I == NCH - 2 and I2 == NCH - 1),
                    skip_group_check=True)

    # --- phase C: epilogue (Ln ACTs batched) ---------------------------
    for r in range(ROWS):
        sig_acc = sig_all[:, r * NCH:(r + 1) * NCH]
        cnt_acc = cnt_all[:, r * NCH:(r + 1) * NCH]
        gm1 = gm1_all[:, r * NCH:(r + 1) * NCH]
        mir = mirs[r]

        # soft Ln argument: 1 + soft_rank
        #   = sig_acc + (128 I - mir_sig) + 0.5 + 1
        s1 = small.tile([P, NCH], F32, tag="s1")
        nc.vector.tensor_tensor(s1[:], sig_acc, mir[:, 0:NCH], op=OP.subtract)
        s2 = small.tile([P, NCH], F32, tag="s2")
        nc.vector.tensor_tensor(s2[:], s1[:], const_soft[:], op=OP.add)
        ln_s = small.tile([P, NCH], F32, tag="ln_s")
        nc.scalar.activation(ln_s[:], s2[:], AF.Ln)
        # ideal Ln argument: hard_rank0 + 2
        #   = cnt_acc + (128 I - mir_cnt) + 2
        s3 = small.tile([P, NCH], F32, tag="s3")
        nc.vector.tensor_tensor(s3[:], cnt_acc, mir[:, NCH:2 * NCH],
                                op=OP.subtract)
        s4 = small.tile([P, NCH], F32, tag="s4")
        nc.vector.tensor_tensor(s4[:], s3[:], const_ideal[:], op=OP.add)
        ln_i = small.tile([P, NCH], F32, tag="ln_i")
        nc.scalar.activation(ln_i[:], s4[:], AF.Ln)

        inv_s = small.tile([P, NCH], F32, tag="inv_s")
        nc.vector.reciprocal(inv_s[:], ln_s[:])
        inv_i = small.tile([P, NCH], F32, tag="inv_i")
        nc.vector.reciprocal(inv_i[:], ln_i[:])

        prod_a = small.tile([P, NCH], F32, tag="prod_a")
        nc.vector.tensor_tensor(prod_a[:], gm1, inv_s[:], op=OP.mult)
        nc.vector.reduce_sum(acc_all[:, r:r + 1], prod_a[:],
                             axis=mybir.AxisListType.X)
        prod_b = small.tile([P, NCH], F32, tag="prod_b")
        nc.vector.tensor_tensor(prod_b[:], gm1, inv_i[:], op=OP.mult)
        nc.vector.reduce_sum(acc_all[:, ROWS + r:ROWS + r + 1], prod_b[:],
                             axis=mybir.AxisListType.X)

    if dbg is not None:
        nc.sync.dma_start(dbg["sig"][:, :], sig_all[:])
        nc.sync.dma_start(dbg["cnt"][:, :], cnt_all[:])
        for r in range(ROWS):
            mcopy = small.tile([P, 2 * NCH], F32, tag=f"mcopy{r}")
            nc.vector.tensor_copy(mcopy[:], mirs[r][:])
            nc.sync.dma_start(dbg["mir"][:, r * 2 * NCH:(r + 1) * 2 * NCH],
                              mcopy[:])
        nc.sync.dma_start(dbg["acc"][:, :], acc_all[:])

    # partition-reduce the per-partition partial sums: [128, R] -> [R, 1]
    # (lhsT = acc columns so M = ROWS; an M=1 ones-lhsT matmul is avoided)
    num_red = psum_red.tile([ROWS, 1], F32, tag="num_red")
    nc.tensor.matmul(num_red[:], lhsT=acc_all[:, 0:ROWS], rhs=ones_col[:],
                     start=True, stop=True)
    den_red = psum_red.tile([ROWS, 1], F32, tag="den_red")
    nc.tensor.matmul(den_red[:], lhsT=acc_all[:, ROWS:2 * ROWS],
                     rhs=ones_col[:], start=True, stop=True)

    num_sb = small.tile([ROWS, 1], F32, tag="num_sb")
    nc.vector.tensor_copy(num_sb[:], num_red[:])
    den_sb = small.tile([ROWS, 1], F32, tag="den_sb")
    nc.vector.tensor_copy(den_sb[:], den_red[:])
    inv_den = small.tile([ROWS, 1], F32, tag="inv_den")
    nc.vector.reciprocal(inv_den[:], den_sb[:])
    ratio = small.tile([ROWS, 1], F32, tag="ratio")
    nc.vector.tensor_tensor(ratio[:], num_sb[:], inv_den[:], op=OP.mult)
    rowloss = small.tile([ROWS, 1], F32, tag="rowloss")
    nc.vector.tensor_scalar(rowloss[:], ratio[:], -1.0, 1.0,
                            op0=OP.mult, op1=OP.add)
    nc.sync.dma_start(out[:, :], rowloss[:])


def build(debug: bool = False) -> bass.Bass:
    nc = bacc.Bacc(trn_type="TRN2")
    pred = nc.dram_tensor("predictions", [ROWS, L], F32, kind="ExternalInput")
    targ = nc.dram_tensor("targets", [ROWS, L], F32, kind="ExternalInput")
    out = nc.dram_tensor("out", [ROWS, 1], F32, kind="ExternalOutput")
    dbg = None
    if debug:
        dbg = {
            "sig": nc.dram_tensor("dbg_sig", [P, NCH * ROWS], F32,
                                  kind="ExternalOutput").ap(),
            "cnt": nc.dram_tensor("dbg_cnt", [P, NCH * ROWS], F32,
                                  kind="ExternalOutput").ap(),
            "mir": nc.dram_tensor("dbg_mir", [P, 2 * NCH * ROWS], F32,
                                  kind="ExternalOutput").ap(),
            "acc": nc.dram_tensor("dbg_acc", [P, 2 * ROWS], F32,
                                  kind="ExternalOutput").ap(),
        }
    with tile.TileContext(nc) as tc:
        with ExitStack() as ctx:
            _emit(ctx, tc, pred.ap(), targ.ap(), out.ap(), dbg)
    nc.compile()
    return nc


def make_in_maps(predictions: np.ndarray, targets: np.ndarray):
    predictions = np.ascontiguousarray(predictions, dtype=np.float32)
    targets = np.ascontiguousarray(targets, dtype=np.float32)
    return [
        {
            "predictions": predictions[c * ROWS:(c + 1) * ROWS],
            "targets": targets[c * ROWS:(c + 1) * ROWS],
        }
        for c in range(NCORES)
    ]


def kernel(predictions: np.ndarray, targets: np.ndarray, _trace: bool = False,
           **_run_kwargs):
    nc = build()
    in_maps = make_in_maps(predictions, targets)
    res = run_bass_kernel_spmd(nc, in_maps, core_ids=list(range(NCORES)),
                               trace=_trace, **_run_kwargs)
    partial = sum(float(r["out"][:, 0].sum()) for r in res.results)
    loss = np.float32(partial / B)
    if _trace:
        return np.asarray(loss), res
    return np.asarray(loss)
